# revision 1
# baseline (speedup 1.0000x reference)
"""Trainium2 Bass kernel for a 4-layer GQA transformer LM (nn_CustomLLM_35278861369705).

Sharding: sequence-parallel across 8 cores — 2 batch groups x 4 sequence chunks
of 256 tokens. Activations kept transposed [feature, token] on device.
Per layer: RMSNorm (ones-matmul partition reduction), fused-rope QKV,
group-local AllGather of K/V, masked full-kv attention (uniform SPMD program;
per-core mask data), SwiGLU MLP with PSUM-resident down-proj accumulators.
Final AllGather of hidden states + vocab-sharded tied LM head.
All matmuls run as float32r (full-rate fp32 storage, ~1e-4 rounding).
"""
import numpy as np

import concourse.bass as bass
import concourse.mybir as mybir
import concourse.tile as tile
from concourse import bacc
from concourse.bass_utils import run_bass_kernel_spmd

V, H, NH, KVH, I, L, S, B = 32000, 1024, 16, 4, 4096, 4, 1024, 2
HD = 64
THETA = 10000.0
EPS = 1e-5
T = 256            # tokens per core
NCORE = 8
GROUPS = [[0, 1, 2, 3], [4, 5, 6, 7]]
VSH = V // 4       # vocab shard per core (within its 4-core group)
KT = H // 128      # 8
IT = I // 128      # 32
NVC = 16           # vocab chunks per core
VC = VSH // NVC    # 500

F32 = mybir.dt.float32
F32R = mybir.dt.float32r
AF = mybir.ActivationFunctionType

_CACHE = {}


def build_program(debug_layers=False, single_core=False, repeat=1):
    nc = bacc.Bacc("TRN2", target_bir_lowering=False, debug=False,
                   num_devices=1 if single_core else NCORE)

    # ---------------- I/O ----------------
    x0T = nc.dram_tensor("x0T", [H, T], F32, kind="ExternalInput").ap()
    cos2 = nc.dram_tensor("cos2", [128, T], F32, kind="ExternalInput").ap()
    sin2 = nc.dram_tensor("sin2", [128, T], F32, kind="ExternalInput").ap()
    ropeR = nc.dram_tensor("ropeR", [128, 128], F32, kind="ExternalInput").ap()
    ones_in = nc.dram_tensor("ones_in", [128, 128], F32, kind="ExternalInput").ap()
    mask_in = nc.dram_tensor("mask", [8, 128, T], F32, kind="ExternalInput").ap()
    embT = nc.dram_tensor("embT", [H, VSH], F32, kind="ExternalInput").ap()
    wq_d, wk_d, wv_d, wo_d, wg_d, wu_d, wd_d = [], [], [], [], [], [], []
    for l in range(L):
        wq_d.append(nc.dram_tensor(f"wq{l}", [H, H], F32, kind="ExternalInput").ap())
        wk_d.append(nc.dram_tensor(f"wk{l}", [H, KVH * HD], F32, kind="ExternalInput").ap())
        wv_d.append(nc.dram_tensor(f"wv{l}", [H, KVH * HD], F32, kind="ExternalInput").ap())
        wo_d.append(nc.dram_tensor(f"wo{l}", [H, H], F32, kind="ExternalInput").ap())
        wg_d.append(nc.dram_tensor(f"wg{l}", [H, I], F32, kind="ExternalInput").ap())
        wu_d.append(nc.dram_tensor(f"wu{l}", [H, I], F32, kind="ExternalInput").ap())
        wd_d.append(nc.dram_tensor(f"wd{l}", [I, H], F32, kind="ExternalInput").ap())
    logits = nc.dram_tensor("logits", [S, VSH], F32, kind="ExternalOutput").ap()
    dbg = []
    dbgs = {}
    if debug_layers:
        for l in range(L):
            dbg.append(nc.dram_tensor(f"dbg_x{l}", [H, T], F32, kind="ExternalOutput").ap())
        dbgs["h"] = nc.dram_tensor("dbg_h", [H, T], F32, kind="ExternalOutput").ap()
        dbgs["q"] = nc.dram_tensor("dbg_q", [H, T], F32, kind="ExternalOutput").ap()
        dbgs["k"] = nc.dram_tensor("dbg_k", [256, T], F32, kind="ExternalOutput").ap()
        dbgs["v"] = nc.dram_tensor("dbg_v", [256, T], F32, kind="ExternalOutput").ap()
        dbgs["o"] = nc.dram_tensor("dbg_o", [H, T], F32, kind="ExternalOutput").ap()
        dbgs["xa"] = nc.dram_tensor("dbg_xa", [H, T], F32, kind="ExternalOutput").ap()

    _dma_rr = [0]

    def wdma(dst, srcap):
        eng = (nc.sync, nc.scalar)[_dma_rr[0] % 2]
        _dma_rr[0] += 1
        eng.dma_start(dst, srcap)

    with tile.TileContext(nc) as tc:
        with (
            tc.tile_pool(name="const", bufs=1) as cpool,
            tc.tile_pool(name="xres", bufs=1) as xpool,
            tc.tile_pool(name="hnorm", bufs=2) as hpool,
            tc.tile_pool(name="sqp", bufs=1) as sqpool,
            tc.tile_pool(name="tmps", bufs=3) as tpool,
            tc.tile_pool(name="dram", bufs=2, space="DRAM") as dpool,
        ):
            # ---- persistent constants ----
            cos_sb = cpool.tile([128, T], F32, tag="cos")
            sin_sb = cpool.tile([128, T], F32, tag="sin")
            nc.sync.dma_start(cos_sb[:], cos2[:])
            nc.sync.dma_start(sin_sb[:], sin2[:])
            ropeR_sb = cpool.tile([128, 128], F32R, tag="ropeR")
            nc.sync.dma_start(ropeR_sb[:], ropeR.bitcast(F32R))
            ones_sb = cpool.tile([128, 128], F32R, tag="ones")
            nc.sync.dma_start(ones_sb[:], ones_in.bitcast(F32R))
            mask_sb = cpool.tile([128, 8, T], F32, tag="mask")
            nc.sync.dma_start(mask_sb[:], mask_in.rearrange("j p t -> p j t"))

            # ---- residual stream ----
            xT = xpool.tile([128, KT, T], F32, tag="xT")
            nc.sync.dma_start(xT[:], x0T.rearrange("(kt p) t -> p kt t", p=128))

            def rmsnorm(src):
                """src: [128, KT, T] f32 -> hT [128, KT, T] f32r (no norm weight:
                weights are folded into the following matmul weights on host)."""
                sq = sqpool.tile([128, KT, T], F32R, tag="sq")
                nc.vector.tensor_mul(out=sq[:], in0=src[:], in1=src[:])
                with tc.tile_pool(name="psnorm", bufs=1, space="PSUM") as pp:
                    ps = pp.tile([128, T], F32, tag="ps_norm")
                    for kt in range(KT):
                        nc.tensor.matmul(ps[:], ones_sb[:], sq[:, kt],
                                         start=(kt == 0), stop=(kt == KT - 1))
                    ms = tpool.tile([128, T], F32, tag="ms")
                    nc.scalar.activation(ms[:], ps[:], AF.Copy, bias=EPS, scale=1.0 / H)
                rcp = tpool.tile([128, T], F32, tag="rcp")
                nc.vector.reciprocal(rcp[:], ms[:])
                inv = tpool.tile([128, T], F32, tag="inv")
                nc.scalar.activation(inv[:], rcp[:], AF.Sqrt)
                hT = hpool.tile([128, KT, T], F32R, tag="h")
                nc.vector.tensor_mul(out=hT[:], in0=src[:],
                                     in1=inv[:, None, :].to_broadcast((128, KT, T)))
                return hT

            # =================== layers ===================
            layer_scope = (
                tc.tile_pool(name="acts", bufs=1),
                tc.tile_pool(name="weights", bufs=8),
                tc.tile_pool(name="wop", bufs=2),
                tc.tile_pool(name="wrhs", bufs=2),
            )
            apool, wpool, wopool, wrpool = [p.__enter__() for p in layer_scope]
            for li in range(L * repeat):
                l = li % L
                with nc.named_scope(f"layer{li}_qkv"):
                    hT = rmsnorm(xT)
                    qT = apool.tile([128, KT, T], F32R, tag="qT")
                    kT_loc = apool.tile([128, 2, T], F32, tag="kT_loc")
                    v_loc = apool.tile([128, 2, T], F32, tag="v_loc")

                    with tc.tile_pool(name="psqkv", bufs=2, space="PSUM") as pq:
                        def proj_rope(w_dram, m, out_sl):
                            """project one 128-feature slice and apply rope into out_sl."""
                            wt = wpool.tile([128, KT, 128], F32R, tag="w_h")
                            wsrc = w_dram.rearrange("(kt p) f -> p kt f", p=128)
                            for hh_ in range(2):
                                wdma(wt[:, hh_ * 4:(hh_ + 1) * 4, :],
                                     wsrc[:, hh_ * 4:(hh_ + 1) * 4,
                                          m * 128:(m + 1) * 128].bitcast(F32R))
                            ps = pq.tile([128, T], F32, tag="ps_qkv")
                            for kt in range(KT):
                                nc.tensor.matmul(ps[:], wt[:, kt], hT[:, kt],
                                                 start=(kt == 0), stop=(kt == KT - 1))
                            raw = tpool.tile([128, T], F32R, tag="qraw")
                            nc.vector.tensor_copy(out=raw[:], in_=ps[:])
                            rot = pq.tile([128, T], F32, tag="ps_rot")
                            nc.tensor.matmul(rot[:], ropeR_sb[:], raw[:],
                                             start=True, stop=True)
                            tcs = tpool.tile([128, T], F32, tag="tcos")
                            nc.vector.tensor_mul(out=tcs[:], in0=ps[:], in1=cos_sb[:])
                            tsn = tpool.tile([128, T], F32, tag="tsin")
                            nc.vector.tensor_mul(out=tsn[:], in0=rot[:], in1=sin_sb[:])
                            nc.vector.tensor_add(out=out_sl, in0=tcs[:], in1=tsn[:])

                        for m in range(KT):
                            proj_rope(wq_d[l], m, qT[:, m, :])
                        for m in range(2):
                            proj_rope(wk_d[l], m, kT_loc[:, m, :])
                        # v: natural layout [tok, feat]
                        wvt = wrpool.tile([128, KT, 256], F32R, tag="w_v")
                        for hh_ in range(2):
                            wdma(wvt[:, hh_ * 4:(hh_ + 1) * 4, :],
                                 wv_d[l].rearrange("(kt p) f -> p kt f", p=128)
                                 [:, hh_ * 4:(hh_ + 1) * 4, :].bitcast(F32R))
                        for tt in range(2):
                            psv = pq.tile([128, 256], F32, tag="ps_v")
                            for kt in range(KT):
                                nc.tensor.matmul(psv[:], hT[:, kt, tt * 128:(tt + 1) * 128],
                                                 wvt[:, kt],
                                                 start=(kt == 0), stop=(kt == KT - 1))
                            nc.vector.tensor_copy(out=v_loc[:, tt, :], in_=psv[:])

                    if debug_layers and li == 0:
                        nc.sync.dma_start(
                            dbgs["h"].rearrange("(kt p) t -> p kt t", p=128),
                            hT.bitcast(F32))
                        nc.sync.dma_start(
                            dbgs["q"].rearrange("(kt p) t -> p kt t", p=128),
                            qT.bitcast(F32))
                        nc.sync.dma_start(
                            dbgs["k"].rearrange("(s p) t -> p s t", p=128), kT_loc[:])
                        nc.sync.dma_start(
                            dbgs["v"].rearrange("(s p) t -> p s t", p=128), v_loc[:])

                    # ---- AllGather K/V within group ----
                    cc_in = dpool.tile([4 * 128, T], F32, tag="cc_in")
                    cc_in_r = cc_in.rearrange("(s p) t -> p s t", p=128)
                    nc.sync.dma_start(cc_in_r[:, 0:2, :], kT_loc[:])
                    nc.sync.dma_start(cc_in_r[:, 2:4, :], v_loc[:])
                    cc_out = dpool.tile([4 * 4 * 128, T], F32, tag="cc_out")
                    if single_core:
                        for rr in range(4):
                            nc.sync.dma_start(cc_out[rr * 512:(rr + 1) * 512], cc_in[:])
                    else:
                        nc.gpsimd.collective_compute(
                            "AllGather", mybir.AluOpType.bypass,
                            ins=[cc_in.opt()], outs=[cc_out.opt()],
                            replica_groups=GROUPS)
                    cc_r = cc_out.rearrange("(c s p) t -> p c s t", c=4, s=4)

                with nc.named_scope(f"layer{li}_attn"):
                    kg = []
                    vg = []
                    for g in range(KVH):
                        # duplicate k rows into both partition halves so matmuls
                        # with q heads at base 0 or 64 both have matching bases
                        kgt = apool.tile([128, 4, T], F32R, tag=f"kg{g}")
                        src = cc_r[64 * (g % 2):64 * (g % 2) + 64, :, g // 2, :].bitcast(F32R)
                        nc.sync.dma_start(kgt[0:64], src)
                        nc.sync.dma_start(kgt[64:128], src)
                        kg.append(kgt)
                        vgt = apool.tile([128, 4, 2, HD], F32R, tag=f"vg{g}")
                        for tt in range(2):
                            nc.sync.dma_start(
                                vgt[:, :, tt, :],
                                cc_r[:, :, 2 + tt, g * HD:(g + 1) * HD].bitcast(F32R))
                        vg.append(vgt)

                    oT = apool.tile([64, NH, T], F32R, tag="oT")
                    with (
                        tc.tile_pool(name="psatt", bufs=2, space="PSUM") as pa,
                        tc.tile_pool(name="pexp", bufs=4) as epool,
                    ):
                        for h in range(NH):
                            g = h // 4
                            q_sl = qT[64 * (h % 2):64 * (h % 2) + 64, h // 2, :]
                            base = 64 * (h % 2)
                            pjs = []
                            for c in range(4):
                                ps_s = pa.tile([128, 2, T], F32, tag="ps_s")
                                for mt in range(2):
                                    nc.tensor.matmul(
                                        ps_s[:, mt, :],
                                        kg[g][base:base + 64, c, mt * 128:(mt + 1) * 128],
                                        q_sl, start=True, stop=True)
                                e1 = epool.tile([128, 2, T], F32, tag="e1")
                                nc.scalar.activation(e1[:], ps_s[:], AF.Exp, scale=0.125)
                                pj = epool.tile([128, 2, T], F32R, tag="pj")
                                nc.vector.tensor_mul(out=pj[:], in0=e1[:],
                                                     in1=mask_sb[:, 2 * c:2 * c + 2, :])
                                pjs.append(pj)
                            ps_sum = pa.tile([128, T], F32, tag="ps_sum")
                            ps_o = pa.tile([64, T], F32, tag="ps_o")
                            for c in range(4):
                                for tt in range(2):
                                    j = 2 * c + tt
                                    nc.tensor.matmul(ps_sum[:], ones_sb[:],
                                                     pjs[c][:, tt, :],
                                                     start=(j == 0), stop=(j == 7))
                                    nc.tensor.matmul(ps_o[:], vg[g][:, c, tt, :],
                                                     pjs[c][:, tt, :],
                                                     start=(j == 0), stop=(j == 7))
                            invb = epool.tile([128, T], F32, tag="invb")
                            nc.vector.reciprocal(invb[:], ps_sum[:])
                            nc.vector.tensor_mul(out=oT[:, h, :], in0=ps_o[:],
                                                 in1=invb[0:64, :])

                    # ---- o-projection + residual ----
                    wo_r = wo_d[l].rearrange("(hh p) f -> p hh f", p=64)
                    with tc.tile_pool(name="psoproj", bufs=2, space="PSUM") as po:
                        for m in range(KT):
                            wot = wopool.tile([64, NH, 128], F32R, tag="w_o")
                            for hh_ in range(2):
                                wdma(wot[:, hh_ * 8:(hh_ + 1) * 8, :],
                                     wo_r[:, hh_ * 8:(hh_ + 1) * 8,
                                          m * 128:(m + 1) * 128].bitcast(F32R))
                            ps = po.tile([128, T], F32, tag="ps_op")
                            for hh in range(NH):
                                nc.tensor.matmul(ps[:], wot[:, hh], oT[:, hh, :],
                                                 start=(hh == 0), stop=(hh == NH - 1))
                            nc.vector.tensor_add(out=xT[:, m, :], in0=xT[:, m, :], in1=ps[:])

                    if debug_layers and li == 0:
                        nc.sync.dma_start(
                            dbgs["o"].rearrange("(hh p) t -> p hh t", p=64),
                            oT.bitcast(F32))
                        nc.sync.dma_start(
                            dbgs["xa"].rearrange("(kt p) t -> p kt t", p=128), xT[:])

                with nc.named_scope(f"layer{li}_mlp"):
                    h2T = rmsnorm(xT)
                    with (
                        tc.tile_pool(name="psmlpd", bufs=1, space="PSUM") as pmd,
                        tc.tile_pool(name="psmlp", bufs=2, space="PSUM") as pm,
                    ):
                        ps_d = [pmd.tile([128, 2, T], F32, tag=f"ps_d{i}", name=f"ps_d{i}")
                                for i in range(4)]
                        for f in range(IT):
                            wgt = wpool.tile([128, KT, 128], F32R, tag="w_h")
                            for hh_ in range(2):
                                wdma(wgt[:, hh_ * 4:(hh_ + 1) * 4, :],
                                     wg_d[l].rearrange("(kt p) f -> p kt f", p=128)
                                     [:, hh_ * 4:(hh_ + 1) * 4,
                                      f * 128:(f + 1) * 128].bitcast(F32R))
                            ps_g = pm.tile([128, T], F32, tag="ps_g")
                            for kt in range(KT):
                                nc.tensor.matmul(ps_g[:], wgt[:, kt], h2T[:, kt],
                                                 start=(kt == 0), stop=(kt == KT - 1))
                            wut = wpool.tile([128, KT, 128], F32R, tag="w_h")
                            for hh_ in range(2):
                                wdma(wut[:, hh_ * 4:(hh_ + 1) * 4, :],
                                     wu_d[l].rearrange("(kt p) f -> p kt f", p=128)
                                     [:, hh_ * 4:(hh_ + 1) * 4,
                                      f * 128:(f + 1) * 128].bitcast(F32R))
                            ps_u = pm.tile([128, T], F32, tag="ps_u")
                            for kt in range(KT):
                                nc.tensor.matmul(ps_u[:], wut[:, kt], h2T[:, kt],
                                                 start=(kt == 0), stop=(kt == KT - 1))
                            silu = tpool.tile([128, T], F32, tag="silu")
                            nc.scalar.activation(silu[:], ps_g[:], AF.Silu)
                            gu = tpool.tile([128, T], F32R, tag="gu")
                            nc.vector.tensor_mul(out=gu[:], in0=silu[:], in1=ps_u[:])
                            wdt = wpool.tile([128, KT, 128], F32R, tag="w_h")
                            wdsrc = wd_d[l].rearrange("(ft p) f -> p ft f", p=128)[:, f, :]
                            for hh_ in range(2):
                                wdma(wdt[:, hh_ * 4:(hh_ + 1) * 4, :],
                                     wdsrc[:, hh_ * 512:(hh_ + 1) * 512]
                                     .rearrange("p (a b) -> p a b", a=4).bitcast(F32R))
                            for m in range(KT):
                                # start=True clears the WHOLE bank's has_written,
                                # so only the first matmul touching each bank may
                                # set it; the odd slice's first write then stores
                                # (has_written=0) and later writes accumulate.
                                nc.tensor.matmul(ps_d[m // 2][:, m % 2, :],
                                                 wdt[:, m], gu[:],
                                                 start=(f == 0 and m % 2 == 0),
                                                 stop=(f == IT - 1),
                                                 skip_group_check=True)
                        for m in range(KT):
                            nc.vector.tensor_add(out=xT[:, m, :], in0=xT[:, m, :],
                                                 in1=ps_d[m // 2][:, m % 2, :])
                if debug_layers and repeat == 1:
                    nc.sync.dma_start(
                        dbg[l].rearrange("(kt p) t -> p kt t", p=128), xT[:])

            for p in reversed(layer_scope):
                p.__exit__(None, None, None)

            # =================== LM head ===================
            with nc.named_scope("lm_head"):
                hfT = rmsnorm(xT)
                cc2_in = dpool.tile([H, T], F32, tag="cc2_in")
                nc.sync.dma_start(cc2_in.rearrange("(kt p) t -> p kt t", p=128),
                                  hfT.bitcast(F32))
                cc2_out = dpool.tile([4 * H, T], F32, tag="cc2_out")
                if single_core:
                    for rr in range(4):
                        nc.sync.dma_start(cc2_out[rr * H:(rr + 1) * H], cc2_in[:])
                else:
                    nc.gpsimd.collective_compute(
                        "AllGather", mybir.AluOpType.bypass,
                        ins=[cc2_in.opt()], outs=[cc2_out.opt()],
                        replica_groups=GROUPS)
                cc2_r = cc2_out.rearrange("(c kt p) t -> p c kt t", c=4, kt=KT)

                with (
                    tc.tile_pool(name="hall", bufs=1) as hallp,
                    tc.tile_pool(name="embp", bufs=2) as embp,
                    tc.tile_pool(name="lsbp", bufs=4) as lsbp,
                    tc.tile_pool(name="pslm", bufs=4, space="PSUM") as plm,
                ):
                    ha = []
                    for m8 in range(8):
                        hat = hallp.tile([128, KT, 128], F32R, tag=f"ha{m8}")
                        nc.sync.dma_start(
                            hat[:],
                            cc2_r[:, m8 // 2, :, 128 * (m8 % 2):128 * (m8 % 2) + 128]
                            .bitcast(F32R))
                        ha.append(hat)
                    embT_r = embT.rearrange("(kt p) v -> p kt v", p=128)
                    for vc in range(NVC):
                        et = embp.tile([128, KT, VC], F32R, tag="emb")
                        for kt_ in range(KT):
                            wdma(et[:, kt_, :],
                                 embT_r[:, kt_, vc * VC:(vc + 1) * VC].bitcast(F32R))
                        for m8 in range(8):
                            ps = plm.tile([128, VC], F32, tag="ps_lm")
                            for kt in range(KT):
                                nc.tensor.matmul(ps[:], ha[m8][:, kt], et[:, kt],
                                                 start=(kt == 0), stop=(kt == KT - 1))
                            lsb = lsbp.tile([128, VC], F32, tag="lsb")
                            nc.any.tensor_copy(out=lsb[:], in_=ps[:])
                            nc.sync.dma_start(
                                logits[m8 * 128:(m8 + 1) * 128, vc * VC:(vc + 1) * VC],
                                lsb[:])

    nc.finalize()
    return nc


# ---------------- host side ----------------

def _host_prep(inputs):
    """Build per-core input maps from full inputs."""
    ids = np.asarray(inputs["input_ids"])
    embed = np.asarray(inputs["embed"], dtype=np.float32)
    n1 = np.asarray(inputs["norm1_w"], dtype=np.float32)
    n2 = np.asarray(inputs["norm2_w"], dtype=np.float32)
    nf = np.asarray(inputs["final_norm_w"], dtype=np.float32)

    inv_freq = 1.0 / (THETA ** (np.arange(0, HD, 2, dtype=np.float64) / HD))
    R64 = np.zeros((HD, HD), np.float32)
    for i in range(32):
        R64[i, i + 32] = -1.0
        R64[i + 32, i] = 1.0
    Rblk = np.zeros((128, 128), np.float32)
    Rblk[:64, :64] = R64
    Rblk[64:, 64:] = R64
    ropeR = np.ascontiguousarray(Rblk.T)
    ones128 = np.ones((128, 128), np.float32)

    # fold norm weights into following matmul weights (they are ones in practice,
    # but fold anyway for generality)
    common = {"ropeR": ropeR, "ones_in": ones128}
    for l in range(L):
        common[f"wq{l}"] = np.ascontiguousarray(n1[l][:, None] * np.asarray(inputs["wq"][l], np.float32))
        common[f"wk{l}"] = np.ascontiguousarray(n1[l][:, None] * np.asarray(inputs["wk"][l], np.float32))
        common[f"wv{l}"] = np.ascontiguousarray(n1[l][:, None] * np.asarray(inputs["wv"][l], np.float32))
        common[f"wo{l}"] = np.ascontiguousarray(np.asarray(inputs["wo"][l], np.float32))
        common[f"wg{l}"] = np.ascontiguousarray(n2[l][:, None] * np.asarray(inputs["w_gate"][l], np.float32))
        common[f"wu{l}"] = np.ascontiguousarray(n2[l][:, None] * np.asarray(inputs["w_up"][l], np.float32))
        common[f"wd{l}"] = np.ascontiguousarray(np.asarray(inputs["w_down"][l], np.float32))

    in_maps = []
    for core in range(NCORE):
        b, qc = core // 4, core % 4
        pos = np.arange(T, dtype=np.float64) + qc * T
        freqs = np.outer(pos, inv_freq)
        emb = np.concatenate([freqs, freqs], axis=-1)
        cosT = np.cos(emb).T.astype(np.float32)
        sinT = np.sin(emb).T.astype(np.float32)
        mask = np.zeros((8, 128, T), np.float32)
        kvpos = np.arange(1024).reshape(8, 128)
        qpos = (np.arange(T) + qc * T)
        for j in range(8):
            mask[j] = (kvpos[j][:, None] <= qpos[None, :]).astype(np.float32)
        x0T = np.ascontiguousarray(embed[ids[b, qc * T:(qc + 1) * T]].T)
        vbase = (core % 4) * VSH
        embT_shard = np.ascontiguousarray((nf[:, None] * embed[vbase:vbase + VSH].T))
        m = dict(common)
        m.update({
            "x0T": x0T.astype(np.float32),
            "cos2": np.ascontiguousarray(np.tile(cosT, (2, 1))),
            "sin2": np.ascontiguousarray(np.tile(sinT, (2, 1))),
            "mask": mask,
            "embT": embT_shard.astype(np.float32),
        })
        in_maps.append(m)
    return in_maps


def _get_program(debug_layers=False):
    key = ("prog", debug_layers)
    if key not in _CACHE:
        _CACHE[key] = build_program(debug_layers)
    return _CACHE[key]


def run(inputs, debug_layers=False, trace=False):
    nc = _get_program(debug_layers)
    in_maps = _host_prep(inputs)
    res = run_bass_kernel_spmd(nc, in_maps, core_ids=list(range(NCORE)), trace=trace)
    out = np.zeros((B, S, V), np.float32)
    for b in range(B):
        out[b] = np.concatenate(
            [res.results[4 * b + i]["logits"] for i in range(4)], axis=1)
    return out, res


def kernel(**inputs) -> np.ndarray:
    out, _ = run(inputs)
    return out



# revision 5
# speedup vs baseline: 1.4192x; 1.4192x over previous
"""Trainium2 Bass kernel for a 4-layer GQA transformer LM (nn_CustomLLM_35278861369705).

Sharding: sequence-parallel across 8 cores — 2 batch groups x 4 sequence chunks
of 256 tokens. Activations kept transposed [feature, token] on device.

v2: all matmul operands bf16 (PSUM accumulation fp32, residual stream fp32),
host pre-lays every weight into its exact SBUF tile layout so each weight
matrix is ONE contiguous large DMA (4-32KB/partition descriptors at line
rate), full-layer weights single-buffered and prefetched a layer ahead, MLP
weights streamed in double-buffered 4-ftile chunks, K/V computed before Q so
the group AllGather overlaps Q-projection, and the softmax denominator is
fused into the attention o-matmul via a ones-column appended to V.
"""
import numpy as np
import ml_dtypes

import concourse.bass as bass
import concourse.mybir as mybir
import concourse.tile as tile
from concourse import bacc
from concourse.bass_utils import run_bass_kernel_spmd

V, H, NH, KVH, I, L, S, B = 32000, 1024, 16, 4, 4096, 4, 1024, 2
HD = 64
THETA = 10000.0
EPS = 1e-5
T = 256            # tokens per core
NCORE = 8
GROUPS = [[0, 1, 2, 3], [4, 5, 6, 7]]
VSH = V // 4       # vocab shard per core (within its 4-core group)
KT = H // 128      # 8
IT = I // 128      # 32
NVC = 16           # vocab chunks per core
VC = VSH // NVC    # 500
FCH = 4            # MLP f-tiles per streamed chunk
NCH = IT // FCH    # 8

F32 = mybir.dt.float32
BF = mybir.dt.bfloat16
AF = mybir.ActivationFunctionType
BF_NP = ml_dtypes.bfloat16

_CACHE = {}


def build_program():
    nc = bacc.Bacc("TRN2", target_bir_lowering=False, debug=False,
                   num_devices=NCORE)

    # ---------------- I/O ----------------
    x0 = nc.dram_tensor("x0", [128, KT, T], F32, kind="ExternalInput").ap()
    cos2 = nc.dram_tensor("cos2", [128, T], F32, kind="ExternalInput").ap()
    sin2 = nc.dram_tensor("sin2", [128, T], F32, kind="ExternalInput").ap()
    ropeR = nc.dram_tensor("ropeR", [128, 128], BF, kind="ExternalInput").ap()
    ones_in = nc.dram_tensor("ones_in", [128, 128], BF, kind="ExternalInput").ap()
    mask_in = nc.dram_tensor("mask", [128, 8, T], BF, kind="ExternalInput").ap()
    emb_in = nc.dram_tensor("embT", [128, NVC, KT, VC], BF, kind="ExternalInput").ap()
    wq_d, wk_d, wv_d, wo_d, wg_d, wu_d, wd_d = [], [], [], [], [], [], []
    for l in range(L):
        wq_d.append(nc.dram_tensor(f"wq{l}", [128, KT, KT, 128], BF, kind="ExternalInput").ap())
        wk_d.append(nc.dram_tensor(f"wk{l}", [128, KT, 2, 128], BF, kind="ExternalInput").ap())
        wv_d.append(nc.dram_tensor(f"wv{l}", [128, KT, 256], BF, kind="ExternalInput").ap())
        wo_d.append(nc.dram_tensor(f"wo{l}", [64, NH, KT, 128], BF, kind="ExternalInput").ap())
        wg_d.append(nc.dram_tensor(f"wg{l}", [128, IT, KT, 128], BF, kind="ExternalInput").ap())
        wu_d.append(nc.dram_tensor(f"wu{l}", [128, IT, KT, 128], BF, kind="ExternalInput").ap())
        wd_d.append(nc.dram_tensor(f"wd{l}", [128, IT, KT, 128], BF, kind="ExternalInput").ap())
    logits = nc.dram_tensor("logits", [S, VSH], BF, kind="ExternalOutput").ap()

    with tile.TileContext(nc) as tc:
        with (
            tc.tile_pool(name="const", bufs=1) as cpool,
            tc.tile_pool(name="xres", bufs=1) as xpool,
            tc.tile_pool(name="hnorm", bufs=2) as hpool,
            tc.tile_pool(name="sqp", bufs=1) as sqpool,
            tc.tile_pool(name="tmps", bufs=3) as tpool,
            tc.tile_pool(name="dram", bufs=2, space="DRAM") as dpool,
        ):
            # ---- persistent constants ----
            cos_sb = cpool.tile([128, T], F32, tag="cos")
            sin_sb = cpool.tile([128, T], F32, tag="sin")
            nc.scalar.dma_start(cos_sb[:], cos2[:])
            nc.scalar.dma_start(sin_sb[:], sin2[:])
            ropeR_sb = cpool.tile([128, 128], BF, tag="ropeR")
            nc.scalar.dma_start(ropeR_sb[:], ropeR[:])
            ones_sb = cpool.tile([128, 128], BF, tag="ones")
            nc.scalar.dma_start(ones_sb[:], ones_in[:])
            mask_sb = cpool.tile([128, 8, T], BF, tag="mask")
            nc.scalar.dma_start(mask_sb[:], mask_in[:])
            onesc = cpool.tile([128, 64], F32, tag="onesc")
            nc.vector.memset(onesc[:], 1.0)

            # ---- residual stream ----
            xT = xpool.tile([128, KT, T], F32, tag="xT")
            nc.scalar.dma_start(xT[:], x0[:])

            def rmsnorm(src):
                """src: [128, KT, T] f32 -> hT [128, KT, T] bf16 (norm weights are
                folded into the following matmul weights on host)."""
                sq = sqpool.tile([128, KT, T], BF, tag="sq")
                nc.vector.tensor_mul(out=sq[:], in0=src[:], in1=src[:])
                with tc.tile_pool(name="psnorm", bufs=1, space="PSUM") as pp:
                    ps = pp.tile([128, T], F32, tag="ps_norm")
                    for kt in range(KT):
                        nc.tensor.matmul(ps[:], ones_sb[:], sq[:, kt],
                                         start=(kt == 0), stop=(kt == KT - 1))
                    ms = tpool.tile([128, T], F32, tag="ms")
                    nc.scalar.activation(ms[:], ps[:], AF.Copy, bias=EPS, scale=1.0 / H)
                rcp = tpool.tile([128, T], F32, tag="rcp")
                nc.vector.reciprocal(rcp[:], ms[:])
                inv = tpool.tile([128, T], F32, tag="inv")
                nc.scalar.activation(inv[:], rcp[:], AF.Sqrt)
                hT = hpool.tile([128, KT, T], BF, tag="h")
                nc.vector.tensor_mul(out=hT[:], in0=src[:],
                                     in1=inv[:, None, :].to_broadcast((128, KT, T)))
                return hT

            # =================== layers ===================
            layer_scope = (
                tc.tile_pool(name="acts", bufs=1),
                tc.tile_pool(name="wqkv", bufs=1),
                tc.tile_pool(name="wmlp", bufs=2),
            )
            apool, wpool, mpool = [p.__enter__() for p in layer_scope]

            # persistent attention gather tiles (ones column set once)
            kg = []
            vg = []
            for g in range(KVH):
                kg.append(apool.tile([128, 4, T], BF, tag=f"kg{g}", name=f"kg{g}"))
                vgt = apool.tile([128, 4, 2, 65], BF, tag=f"vg{g}", name=f"vg{g}")
                nc.vector.memset(vgt[:, :, :, 64:65], 1.0)
                vg.append(vgt)

            for l in range(L):
                with nc.named_scope(f"layer{l}_qkv"):
                    # full-layer weights: one contiguous DMA each
                    wq_sb = wpool.tile([128, KT, KT, 128], BF, tag="wq")
                    wk_sb = wpool.tile([128, KT, 2, 128], BF, tag="wk")
                    wv_sb = wpool.tile([128, KT, 256], BF, tag="wv")
                    wo_sb = wpool.tile([64, NH, KT, 128], BF, tag="wo")
                    nc.scalar.dma_start(wk_sb[:], wk_d[l][:])
                    nc.scalar.dma_start(wv_sb[:], wv_d[l][:])
                    nc.scalar.dma_start(wq_sb[:], wq_d[l][:])
                    nc.scalar.dma_start(wo_sb[:], wo_d[l][:])

                    hT = rmsnorm(xT)
                    qT = apool.tile([128, KT, T], BF, tag="qT")
                    kT_loc = apool.tile([128, 2, T], BF, tag="kT_loc")
                    v_loc = apool.tile([128, 2, T], BF, tag="v_loc")

                    with tc.tile_pool(name="psqkv", bufs=2, space="PSUM") as pq:
                        def proj_rope(w_sl, out_sl):
                            """project one 128-feature slice and apply rope."""
                            ps = pq.tile([128, T], F32, tag="ps_qkv")
                            for kt in range(KT):
                                nc.tensor.matmul(ps[:], w_sl[:, kt], hT[:, kt],
                                                 start=(kt == 0), stop=(kt == KT - 1))
                            raw = tpool.tile([128, T], BF, tag="qraw")
                            nc.vector.tensor_copy(out=raw[:], in_=ps[:])
                            rot = pq.tile([128, T], F32, tag="ps_rot")
                            nc.tensor.matmul(rot[:], ropeR_sb[:], raw[:],
                                             start=True, stop=True)
                            tcs = tpool.tile([128, T], F32, tag="tcos")
                            nc.vector.tensor_mul(out=tcs[:], in0=ps[:], in1=cos_sb[:])
                            tsn = tpool.tile([128, T], F32, tag="tsin")
                            nc.vector.tensor_mul(out=tsn[:], in0=rot[:], in1=sin_sb[:])
                            nc.vector.tensor_add(out=out_sl, in0=tcs[:], in1=tsn[:])

                        # K first, then V, so the AllGather can start early
                        for m in range(2):
                            proj_rope(wk_sb[:, :, m, :], kT_loc[:, m, :])
                        for tt in range(2):
                            psv = pq.tile([128, 256], F32, tag="ps_v")
                            for kt in range(KT):
                                nc.tensor.matmul(psv[:], hT[:, kt, tt * 128:(tt + 1) * 128],
                                                 wv_sb[:, kt],
                                                 start=(kt == 0), stop=(kt == KT - 1))
                            nc.vector.tensor_copy(out=v_loc[:, tt, :], in_=psv[:])

                        # ---- AllGather K/V within group ----
                        cc_in = dpool.tile([4 * 128, T], BF, tag="cc_in")
                        cc_in_r = cc_in.rearrange("(s p) t -> p s t", p=128)
                        nc.sync.dma_start(cc_in_r[:, 0:2, :], kT_loc[:])
                        nc.sync.dma_start(cc_in_r[:, 2:4, :], v_loc[:])
                        cc_out = dpool.tile([4 * 4 * 128, T], BF, tag="cc_out")
                        nc.gpsimd.collective_compute(
                            "AllGather", mybir.AluOpType.bypass,
                            ins=[cc_in.opt()], outs=[cc_out.opt()],
                            replica_groups=GROUPS)
                        cc_r = cc_out.rearrange("(c s p) t -> p c s t", c=4, s=4)

                        # Q projection overlaps the collective
                        for m in range(KT):
                            proj_rope(wq_sb[:, :, m, :], qT[:, m, :])

                    # load gathered K/V (k rows duplicated into both halves so
                    # matmuls with q heads at base 0 or 64 have matching bases)
                    for g in range(KVH):
                        src = cc_r[64 * (g % 2):64 * (g % 2) + 64, :, g // 2, :]
                        nc.scalar.dma_start(kg[g][0:64], src)
                        nc.scalar.dma_start(kg[g][64:128], src)
                        for tt in range(2):
                            nc.scalar.dma_start(
                                vg[g][:, :, tt, 0:64],
                                cc_r[:, :, 2 + tt, g * HD:(g + 1) * HD])

                with nc.named_scope(f"layer{l}_attn"):
                    oT = apool.tile([64, NH, T], BF, tag="oT")
                    with (
                        tc.tile_pool(name="psatt", bufs=2, space="PSUM") as pa,
                        tc.tile_pool(name="pexp", bufs=4) as epool,
                    ):
                        for h in range(NH):
                            g = h // 4
                            base = 64 * (h % 2)
                            q_sl = qT[base:base + 64, h // 2, :]
                            pjs = []
                            for c in range(4):
                                ps_s = pa.tile([128, 2, T], F32, tag="ps_s")
                                for mt in range(2):
                                    nc.tensor.matmul(
                                        ps_s[:, mt, :],
                                        kg[g][base:base + 64, c, mt * 128:(mt + 1) * 128],
                                        q_sl, start=True, stop=True)
                                e1 = epool.tile([128, 2, T], F32, tag="e1")
                                nc.scalar.activation(e1[:], ps_s[:], AF.Exp, scale=0.125)
                                pj = epool.tile([128, 2, T], BF, tag="pj")
                                nc.vector.tensor_mul(out=pj[:], in0=e1[:],
                                                     in1=mask_sb[:, 2 * c:2 * c + 2, :])
                                pjs.append(pj)
                            # fused o + denominator (ones column -> row 64)
                            ps_o = pa.tile([65, T], F32, tag="ps_o")
                            for c in range(4):
                                for tt in range(2):
                                    j = 2 * c + tt
                                    nc.tensor.matmul(ps_o[:], vg[g][:, c, tt, :],
                                                     pjs[c][:, tt, :],
                                                     start=(j == 0), stop=(j == 7))
                            rec = epool.tile([65, T], F32, tag="rec")
                            nc.vector.reciprocal(rec[64:65, :], ps_o[64:65, :])
                            inv_ps = pa.tile([64, T], F32, tag="inv_ps")
                            nc.tensor.matmul(inv_ps[:], onesc[64:65, :],
                                             rec[64:65, :], start=True, stop=True)
                            inv_sb = epool.tile([64, T], F32, tag="inv_sb")
                            nc.scalar.activation(inv_sb[:], inv_ps[:], AF.Copy)
                            nc.vector.tensor_mul(out=oT[:, h, :], in0=ps_o[0:64, :],
                                                 in1=inv_sb[:])

                    # ---- o-projection + residual ----
                    with tc.tile_pool(name="psoproj", bufs=2, space="PSUM") as po:
                        for m in range(KT):
                            ps = po.tile([128, T], F32, tag="ps_op")
                            for hh in range(NH):
                                nc.tensor.matmul(ps[:], wo_sb[:, hh, m, :], oT[:, hh, :],
                                                 start=(hh == 0), stop=(hh == NH - 1))
                            nc.vector.tensor_add(out=xT[:, m, :], in0=xT[:, m, :], in1=ps[:])

                with nc.named_scope(f"layer{l}_mlp"):
                    h2T = rmsnorm(xT)
                    with (
                        tc.tile_pool(name="psmlpd", bufs=1, space="PSUM") as pmd,
                        tc.tile_pool(name="psmlp", bufs=2, space="PSUM") as pm,
                    ):
                        ps_d = [pmd.tile([128, 2, T], F32, tag=f"ps_d{i}", name=f"ps_d{i}")
                                for i in range(4)]
                        for ch in range(NCH):
                            wg_sb = mpool.tile([128, FCH, KT, 128], BF, tag="wg")
                            wu_sb = mpool.tile([128, FCH, KT, 128], BF, tag="wu")
                            wd_sb = mpool.tile([128, FCH, KT, 128], BF, tag="wd")
                            nc.sync.dma_start(wg_sb[:], wg_d[l][:, ch * FCH:(ch + 1) * FCH])
                            nc.sync.dma_start(wu_sb[:], wu_d[l][:, ch * FCH:(ch + 1) * FCH])
                            nc.sync.dma_start(wd_sb[:], wd_d[l][:, ch * FCH:(ch + 1) * FCH])
                            for fi in range(FCH):
                                f = ch * FCH + fi
                                ps_g = pm.tile([128, T], F32, tag="ps_g")
                                for kt in range(KT):
                                    nc.tensor.matmul(ps_g[:], wg_sb[:, fi, kt], h2T[:, kt],
                                                     start=(kt == 0), stop=(kt == KT - 1))
                                ps_u = pm.tile([128, T], F32, tag="ps_u")
                                for kt in range(KT):
                                    nc.tensor.matmul(ps_u[:], wu_sb[:, fi, kt], h2T[:, kt],
                                                     start=(kt == 0), stop=(kt == KT - 1))
                                silu = tpool.tile([128, T], F32, tag="silu")
                                nc.scalar.activation(silu[:], ps_g[:], AF.Silu)
                                gu = tpool.tile([128, T], BF, tag="gu")
                                nc.vector.tensor_mul(out=gu[:], in0=silu[:], in1=ps_u[:])
                                for m in range(KT):
                                    # start=True clears the WHOLE bank's has_written,
                                    # so only the first matmul touching each bank may
                                    # set it; the odd slice's first write then stores
                                    # (has_written=0) and later writes accumulate.
                                    nc.tensor.matmul(ps_d[m // 2][:, m % 2, :],
                                                     wd_sb[:, fi, m], gu[:],
                                                     start=(f == 0 and m % 2 == 0),
                                                     stop=(f == IT - 1),
                                                     skip_group_check=True)
                        for m in range(KT):
                            nc.vector.tensor_add(out=xT[:, m, :], in0=xT[:, m, :],
                                                 in1=ps_d[m // 2][:, m % 2, :])

            for p in reversed(layer_scope):
                p.__exit__(None, None, None)

            # =================== LM head ===================
            with nc.named_scope("lm_head"):
                hfT = rmsnorm(xT)
                cc2_in = dpool.tile([H, T], BF, tag="cc2_in")
                nc.sync.dma_start(cc2_in.rearrange("(kt p) t -> p kt t", p=128),
                                  hfT[:])
                cc2_out = dpool.tile([4 * H, T], BF, tag="cc2_out")
                nc.gpsimd.collective_compute(
                    "AllGather", mybir.AluOpType.bypass,
                    ins=[cc2_in.opt()], outs=[cc2_out.opt()],
                    replica_groups=GROUPS)
                cc2_r = cc2_out.rearrange("(c kt p) t -> p c kt t", c=4, kt=KT)

                with (
                    tc.tile_pool(name="hall", bufs=1) as hallp,
                    tc.tile_pool(name="embp", bufs=2) as embp,
                    tc.tile_pool(name="lsbp", bufs=4) as lsbp,
                    tc.tile_pool(name="pslm", bufs=4, space="PSUM") as plm,
                ):
                    hall = hallp.tile([128, 4, KT, T], BF, tag="hall")
                    nc.scalar.dma_start(hall[:], cc2_r[:])
                    for vc in range(NVC):
                        et = embp.tile([128, KT, VC], BF, tag="emb")
                        nc.sync.dma_start(et[:], emb_in[:, vc])
                        for m8 in range(8):
                            lhs = hall[:, m8 // 2, :, (m8 % 2) * 128:(m8 % 2) * 128 + 128]
                            ps = plm.tile([128, VC], F32, tag="ps_lm")
                            for kt in range(KT):
                                nc.tensor.matmul(ps[:], lhs[:, kt], et[:, kt],
                                                 start=(kt == 0), stop=(kt == KT - 1))
                            lsb = lsbp.tile([128, VC], BF, tag="lsb")
                            nc.any.tensor_copy(out=lsb[:], in_=ps[:])
                            nc.scalar.dma_start(
                                logits[m8 * 128:(m8 + 1) * 128, vc * VC:(vc + 1) * VC],
                                lsb[:])

    nc.finalize()
    return nc


# ---------------- host side ----------------

def _host_prep(inputs):
    """Build per-core input maps from full inputs."""
    ids = np.asarray(inputs["input_ids"])
    embed = np.asarray(inputs["embed"], dtype=np.float32)
    n1 = np.asarray(inputs["norm1_w"], dtype=np.float32)
    n2 = np.asarray(inputs["norm2_w"], dtype=np.float32)
    nf = np.asarray(inputs["final_norm_w"], dtype=np.float32)

    inv_freq = 1.0 / (THETA ** (np.arange(0, HD, 2, dtype=np.float64) / HD))
    R64 = np.zeros((HD, HD), np.float32)
    for i in range(32):
        R64[i, i + 32] = -1.0
        R64[i + 32, i] = 1.0
    Rblk = np.zeros((128, 128), np.float32)
    Rblk[:64, :64] = R64
    Rblk[64:, 64:] = R64
    ropeR = np.ascontiguousarray(Rblk.T).astype(BF_NP)
    ones128 = np.ones((128, 128), BF_NP)

    def prep_lhsT(w, kdim, fdim):
        """[K, F] -> [128, K/128, F/128, 128] tile layout (lhsT slices)."""
        return np.ascontiguousarray(
            w.reshape(kdim // 128, 128, fdim // 128, 128).transpose(1, 0, 2, 3)
        ).astype(BF_NP)

    # fold norm weights into following matmul weights
    common = {"ropeR": ropeR, "ones_in": ones128}
    for l in range(L):
        wq = n1[l][:, None] * np.asarray(inputs["wq"][l], np.float32)
        wk = n1[l][:, None] * np.asarray(inputs["wk"][l], np.float32)
        wv = n1[l][:, None] * np.asarray(inputs["wv"][l], np.float32)
        wo = np.asarray(inputs["wo"][l], np.float32)
        wg = n2[l][:, None] * np.asarray(inputs["w_gate"][l], np.float32)
        wu = n2[l][:, None] * np.asarray(inputs["w_up"][l], np.float32)
        wd = np.asarray(inputs["w_down"][l], np.float32)
        common[f"wq{l}"] = prep_lhsT(wq, H, H)
        common[f"wk{l}"] = prep_lhsT(wk, H, 256)
        # wv is used as matmul RHS: [128, KT, 256]
        common[f"wv{l}"] = np.ascontiguousarray(
            wv.reshape(KT, 128, 256).transpose(1, 0, 2)).astype(BF_NP)
        # wo lhsT slices are [64(d), 128(out)] per (head, m): [64, NH, KT, 128]
        common[f"wo{l}"] = np.ascontiguousarray(
            wo.reshape(NH, 64, KT, 128).transpose(1, 0, 2, 3)).astype(BF_NP)
        # MLP lhsT layouts: [128, f-tile, kt, 128]
        common[f"wg{l}"] = np.ascontiguousarray(
            wg.reshape(KT, 128, IT, 128).transpose(1, 2, 0, 3)).astype(BF_NP)
        common[f"wu{l}"] = np.ascontiguousarray(
            wu.reshape(KT, 128, IT, 128).transpose(1, 2, 0, 3)).astype(BF_NP)
        common[f"wd{l}"] = np.ascontiguousarray(
            wd.reshape(IT, 128, KT, 128).transpose(1, 0, 2, 3)).astype(BF_NP)

    in_maps = []
    for core in range(NCORE):
        b, qc = core // 4, core % 4
        pos = np.arange(T, dtype=np.float64) + qc * T
        freqs = np.outer(pos, inv_freq)
        emb = np.concatenate([freqs, freqs], axis=-1)
        cosT = np.cos(emb).T.astype(np.float32)
        sinT = np.sin(emb).T.astype(np.float32)
        mask = np.zeros((8, 128, T), np.float32)
        kvpos = np.arange(1024).reshape(8, 128)
        qpos = (np.arange(T) + qc * T)
        for j in range(8):
            mask[j] = (kvpos[j][:, None] <= qpos[None, :]).astype(np.float32)
        x0T = embed[ids[b, qc * T:(qc + 1) * T]].T          # [H, T]
        x0p = np.ascontiguousarray(
            x0T.reshape(KT, 128, T).transpose(1, 0, 2)).astype(np.float32)
        vbase = (core % 4) * VSH
        embT_shard = (nf[:, None] * embed[vbase:vbase + VSH].T)   # [H, VSH]
        embp = np.ascontiguousarray(
            embT_shard.reshape(KT, 128, NVC, VC).transpose(1, 2, 0, 3)).astype(BF_NP)
        m = dict(common)
        m.update({
            "x0": x0p,
            "cos2": np.ascontiguousarray(np.tile(cosT, (2, 1))),
            "sin2": np.ascontiguousarray(np.tile(sinT, (2, 1))),
            "mask": np.ascontiguousarray(mask.transpose(1, 0, 2)).astype(BF_NP),
            "embT": embp,
        })
        in_maps.append(m)
    return in_maps


def _get_program():
    if "prog" not in _CACHE:
        _CACHE["prog"] = build_program()
    return _CACHE["prog"]


def run(inputs, debug_layers=False, trace=False):
    nc = _get_program()
    in_maps = _host_prep(inputs)
    res = run_bass_kernel_spmd(nc, in_maps, core_ids=list(range(NCORE)), trace=trace)
    out = np.zeros((B, S, V), np.float32)
    for b in range(B):
        out[b] = np.concatenate(
            [res.results[4 * b + i]["logits"].astype(np.float32) for i in range(4)],
            axis=1)
    return out, res


def kernel(**inputs) -> np.ndarray:
    out, _ = run(inputs)
    return out
